# revision 6
# baseline (speedup 1.0000x reference)
"""Trainium2 Bass kernel for a GQA attention layer (B=2, S=2048, D=4096,
32 q-heads, 8 kv-heads, HD=128, RoPE, causal mask).

Sharding: 8 cores = 2 (batch) x 4 (head groups). Each core handles one
batch and 8 q-heads / 2 kv-heads: column-parallel wq/wk/wv, row-parallel
wo. Each core emits a partial [S, D] output; the host sums the 4 partials
per batch. No collectives.

Device dataflow (per core):
  phase 1: QKV projections from host-pretransposed xT (feature-major),
           RoPE applied in a "split" head layout (host permutes wq/wk
           columns so real/imag parts land in partition halves; the
           cross-partition swap is an SBUF->SBUF DMA).
  phase 2: scoresT[sk,sq] = K^T-tiles (stationary) x Q^T (moving); exp on
           ScalarE with scale=1/sqrt(HD); causal handling = skip fully
           masked sk-tiles + one [128,128] mask-tile add on the diagonal;
           softmax denominator via ones-stationary matmul into psum[1,N];
           attnT accumulated with V (token-major) stationary; 1/denom
           applied during psum evacuation (gpsimd partition_broadcast +
           DVE reciprocal/mul).
  phase 3: out_partial = attnT^T x wo-rows, streamed per 512-col block.
"""

import sys

if "/opt/trn_rl_repo" not in sys.path:
    sys.path.insert(0, "/opt/trn_rl_repo")

import math
from contextlib import ExitStack

import ml_dtypes
import numpy as np

import concourse.bass as bass  # noqa: F401  (AP types used implicitly)
import concourse.tile as tile
from concourse import bacc, mybir
from concourse.bass_utils import run_bass_kernel_spmd

BF16 = ml_dtypes.bfloat16
F32 = mybir.dt.float32
BF = mybir.dt.bfloat16

B, S, D = 2, 2048, 4096
NH, NKV, HD = 32, 8, 128
G = 4  # head groups -> cores per batch
HPG = NH // G  # 8 q heads per core
KPG = NKV // G  # 2 kv heads per core
SCALE = 1.0 / math.sqrt(HD)

NFT = D // 128  # 32 feature tiles (contraction)
PTOK = 512  # token panel width in phase 1
NPANEL = S // PTOK  # 4
NTT = S // 128  # 16 token tiles
NSQ = S // 512  # 4 sq tiles
NOD = D // 512  # 8 out-D tiles

_CACHE = {}


def _build_program(phases=(1, 2, 3)):
    nc = bacc.Bacc("TRN2", target_bir_lowering=False, debug=False, num_devices=8)

    xt = nc.dram_tensor("xt", [D, S], BF, kind="ExternalInput").ap()
    wq = nc.dram_tensor("wq", [NFT, HPG, 128, 128], BF, kind="ExternalInput").ap()
    wk = nc.dram_tensor("wk", [NFT, KPG, 128, 128], BF, kind="ExternalInput").ap()
    wv = nc.dram_tensor("wv", [NFT, 128, KPG * 128], BF, kind="ExternalInput").ap()
    wo = nc.dram_tensor("wo", [HPG, NOD, 128, 512], BF, kind="ExternalInput").ap()
    cosb = nc.dram_tensor("cosb", [128, S], BF, kind="ExternalInput").ap()
    sinb = nc.dram_tensor("sinb", [128, S], BF, kind="ExternalInput").ap()
    diagm = nc.dram_tensor("diagm", [128, 128], F32, kind="ExternalInput").ap()
    ones = nc.dram_tensor("ones", [128, 1], BF, kind="ExternalInput").ap()
    outp = nc.dram_tensor("outp", [S, D], F32, kind="ExternalOutput").ap()

    EXP = mybir.ActivationFunctionType.Exp
    MULT = mybir.AluOpType.mult

    with tile.TileContext(nc) as tc, ExitStack() as ctx:
        pool = lambda name, bufs: ctx.enter_context(tc.tile_pool(name=name, bufs=bufs))
        ppool = lambda name, bufs: ctx.enter_context(
            tc.tile_pool(name=name, bufs=bufs, space="PSUM")
        )

        persist = pool("persist", 1)
        xpool = pool("xpool", NFT + 1)
        wqpool = pool("wqpool", 6)
        wvpool = pool("wvpool", NFT + 2)
        ropepool = pool("ropepool", 6)
        probpool = pool("probpool", 7)
        denpool = pool("denpool", 2)
        bigden = pool("bigden", 2)
        wopool = pool("wopool", 9)
        outpool = pool("outpool", 3)

        psA = ppool("psA", 4)  # [128,512] f32: Q/K proj, scores, O-proj
        psV = ppool("psV", 1)  # [128,256] f32: V proj
        psAt = ppool("psAt", 2)  # [128,512] f32: attn accum
        psD = ppool("psD", 1)  # [1,512] f32: denominators

        # ---- persistent tiles ----
        qt = [persist.tile([128, S], BF, tag=f"qt{h}", name=f"qt{h}") for h in range(HPG)]
        kt = [persist.tile([128, S], BF, tag=f"kt{k}", name=f"kt{k}") for k in range(KPG)]
        v_sb = persist.tile([128, NTT * KPG * 128], BF, tag="v", name="v_sb")
        at = [persist.tile([128, S], BF, tag=f"at{h}", name=f"at{h}") for h in range(HPG)]
        cos_sb = persist.tile([128, S], BF, tag="cos", name="cos_sb")
        sin_sb = persist.tile([128, S], BF, tag="sin", name="sin_sb")
        diag_sb = persist.tile([128, 128], F32, tag="diag", name="diag_sb")
        ones_sb = persist.tile([128, 1], BF, tag="ones", name="ones_sb")

        nc.sync.dma_start(cos_sb[:], cosb[:])
        nc.sync.dma_start(sin_sb[:], sinb[:])
        nc.sync.dma_start(diag_sb[:], diagm[:])
        nc.sync.dma_start(ones_sb[:], ones[:])

        # ================= phase 1: projections + RoPE =================
        do1, do2, do3 = (1 in phases), (2 in phases), (3 in phases)
        def rope_evac(ps, dst, tok0):
            # dst[:, tok0:tok0+512] = ps * C + swap_halves(ps) * S2
            t1 = ropepool.tile([128, 512], F32, tag="rope_t1", name="rope_t1")
            nc.vector.tensor_mul(t1[:], ps[:], cos_sb[:, tok0 : tok0 + 512])
            raw = ropepool.tile([128, 512], BF, tag="rope_raw", name="rope_raw")
            nc.scalar.copy(raw[:], ps[:])
            rsw = ropepool.tile([128, 512], BF, tag="rope_rsw", name="rope_rsw")
            nc.sync.dma_start(rsw[0:64, :], raw[64:128, :])
            nc.sync.dma_start(rsw[64:128, :], raw[0:64, :])
            nc.vector.tensor_mul(rsw[:], rsw[:], sin_sb[:, tok0 : tok0 + 512])
            nc.vector.tensor_add(dst[:, tok0 : tok0 + 512], t1[:], rsw[:])

        for n in range(NPANEL if do1 else 0):
            tok0 = n * PTOK
            xts = []
            for f in range(NFT):
                xtile = xpool.tile([128, PTOK], BF, tag="xts", name="xts")
                nc.sync.dma_start(
                    xtile[:], xt[f * 128 : (f + 1) * 128, tok0 : tok0 + PTOK]
                )
                xts.append(xtile)

            # Q heads
            for h in range(HPG):
                ps = psA.tile([128, 512], F32, tag="psA", name="psA_t")
                for f in range(NFT):
                    wqt = wqpool.tile([128, 128], BF, tag="wqt", name="wqt")
                    nc.sync.dma_start(wqt[:], wq[f, h])
                    nc.tensor.matmul(
                        ps[:], wqt[:], xts[f][:], start=(f == 0), stop=(f == NFT - 1)
                    )
                rope_evac(ps, qt[h], tok0)

            # K heads
            for k in range(KPG):
                ps = psA.tile([128, 512], F32, tag="psA", name="psA_t")
                for f in range(NFT):
                    wkt = wqpool.tile([128, 128], BF, tag="wqt", name="wqt")
                    nc.sync.dma_start(wkt[:], wk[f, k])
                    nc.tensor.matmul(
                        ps[:], wkt[:], xts[f][:], start=(f == 0), stop=(f == NFT - 1)
                    )
                rope_evac(ps, kt[k], tok0)

            # V (token-major, both kv heads side by side)
            wvts = []
            for f in range(NFT):
                wvt = wvpool.tile([128, KPG * 128], BF, tag="wvt", name="wvt")
                nc.sync.dma_start(wvt[:], wv[f])
                wvts.append(wvt)
            for m in range(PTOK // 128):
                ps = psV.tile([128, KPG * 128], F32, tag="psV", name="psV_t")
                for f in range(NFT):
                    nc.tensor.matmul(
                        ps[:],
                        xts[f][:, m * 128 : (m + 1) * 128],
                        wvts[f][:],
                        start=(f == 0),
                        stop=(f == NFT - 1),
                    )
                tglob = n * (PTOK // 128) + m
                nc.scalar.copy(
                    v_sb[:, tglob * 256 : (tglob + 1) * 256], ps[:]
                )

        # ================= phase 2: attention =================
        for h in range(HPG if do2 else 0):
            kv = h // (HPG // KPG)
            for j in range(NSQ):
                n_sk = 4 * (j + 1)
                sq0 = j * 512
                ps_d = psD.tile([1, 512], F32, tag="psD", name="psD_t")
                ps_a = psAt.tile([128, 512], F32, tag="psAt", name="psAt_t")
                for t in range(n_sk):
                    r = t - 4 * j
                    off = 128 * r if r >= 0 else 0
                    ps_s = psA.tile([128, 512], F32, tag="psA", name="psA_t")
                    nc.tensor.matmul(
                        ps_s[:, off:512],
                        kt[kv][:, t * 128 : (t + 1) * 128],
                        qt[h][:, sq0 + off : sq0 + 512],
                        start=True,
                        stop=True,
                    )
                    if r >= 0:
                        nc.vector.tensor_add(
                            ps_s[:, off : off + 128],
                            ps_s[:, off : off + 128],
                            diag_sb[:],
                        )
                    pt = probpool.tile([128, 512], BF, tag="probs", name="probs_t")
                    if off > 0:
                        nc.vector.memset(pt[:, 0:off], 0.0)
                    nc.scalar.activation(
                        pt[:, off:512], ps_s[:, off:512], EXP, scale=SCALE
                    )
                    nc.tensor.matmul(
                        ps_d[:],
                        ones_sb[:],
                        pt[:],
                        start=(t == 0),
                        stop=(t == n_sk - 1),
                    )
                    nc.tensor.matmul(
                        ps_a[:],
                        v_sb[:, t * 256 + kv * 128 : t * 256 + kv * 128 + 128],
                        pt[:],
                        start=(t == 0),
                        stop=(t == n_sk - 1),
                    )
                den_row = denpool.tile([1, 512], F32, tag="den_row", name="den_row")
                nc.scalar.copy(den_row[:], ps_d[:])
                den_b = bigden.tile([128, 512], F32, tag="den_b", name="den_b")
                nc.gpsimd.partition_broadcast(den_b[:], den_row[:])
                inv_b = bigden.tile([128, 512], F32, tag="inv_b", name="inv_b")
                nc.vector.reciprocal(inv_b[:], den_b[:])
                nc.vector.tensor_tensor(
                    at[h][:, sq0 : sq0 + 512], ps_a[:], inv_b[:], MULT
                )

        # ================= phase 3: output projection =================
        for d in range(NOD if do3 else 0):
            wots = []
            for h in range(HPG):
                wot = wopool.tile([128, 512], BF, tag="wot", name="wot")
                nc.sync.dma_start(wot[:], wo[h, d])
                wots.append(wot)
            for m in range(NTT):
                ps = psA.tile([128, 512], F32, tag="psA", name="psA_t")
                for h in range(HPG):
                    nc.tensor.matmul(
                        ps[:],
                        at[h][:, m * 128 : (m + 1) * 128],
                        wots[h][:],
                        start=(h == 0),
                        stop=(h == HPG - 1),
                    )
                osb = outpool.tile([128, 512], F32, tag="osb", name="osb")
                nc.scalar.copy(osb[:], ps[:])
                nc.sync.dma_start(
                    outp[m * 128 : (m + 1) * 128, d * 512 : (d + 1) * 512], osb[:]
                )

    nc.compile()
    return nc


_SPLIT_PERM = np.concatenate([np.arange(0, HD, 2), np.arange(1, HD, 2)])


def _host_prep(x, freqs_cos, freqs_sin, mask, wq, wk, wv, wo):
    """Build per-core input maps (8 cores = 2 batches x 4 head groups)."""
    x = np.asarray(x, np.float32)
    wq = np.asarray(wq, np.float32)
    wk = np.asarray(wk, np.float32)
    wv = np.asarray(wv, np.float32)
    wo = np.asarray(wo, np.float32)
    freqs_cos = np.asarray(freqs_cos, np.float32)
    freqs_sin = np.asarray(freqs_sin, np.float32)
    mask = np.asarray(mask, np.float32)

    xts = [np.ascontiguousarray(x[b].T).astype(BF16) for b in range(B)]

    ct = freqs_cos.T  # [64, S]
    st = freqs_sin.T
    cosb = np.concatenate([ct, ct], axis=0).astype(BF16)
    sinb = np.concatenate([-st, st], axis=0).astype(BF16)
    diagm = np.ascontiguousarray(
        mask[0:128, 0:128].T * math.sqrt(HD), dtype=np.float32
    )
    ones = np.ones((128, 1), BF16)

    per_g = []
    for g in range(G):
        wq_g = wq[:, g * HPG * HD : (g + 1) * HPG * HD].reshape(D, HPG, HD)
        wq_g = wq_g[:, :, _SPLIT_PERM]
        wq_g = np.ascontiguousarray(
            wq_g.reshape(NFT, 128, HPG, 128).transpose(0, 2, 1, 3)
        ).astype(BF16)

        wk_g = wk[:, g * KPG * HD : (g + 1) * KPG * HD].reshape(D, KPG, HD)
        wk_g = wk_g[:, :, _SPLIT_PERM]
        wk_g = np.ascontiguousarray(
            wk_g.reshape(NFT, 128, KPG, 128).transpose(0, 2, 1, 3)
        ).astype(BF16)

        wv_g = np.ascontiguousarray(
            wv[:, g * KPG * HD : (g + 1) * KPG * HD].reshape(NFT, 128, KPG * 128)
        ).astype(BF16)

        wo_g = wo[g * HPG * HD : (g + 1) * HPG * HD, :]
        wo_g = np.ascontiguousarray(
            wo_g.reshape(HPG, 128, NOD, 512).transpose(0, 2, 1, 3)
        ).astype(BF16)

        per_g.append((wq_g, wk_g, wv_g, wo_g))

    in_maps = []
    for core in range(8):
        b, g = divmod(core, G)
        wq_g, wk_g, wv_g, wo_g = per_g[g]
        in_maps.append(
            {
                "xt": xts[b],
                "wq": wq_g,
                "wk": wk_g,
                "wv": wv_g,
                "wo": wo_g,
                "cosb": cosb,
                "sinb": sinb,
                "diagm": diagm,
                "ones": ones,
            }
        )
    return in_maps


def get_program(phases=(1, 2, 3)):
    key = ("nc", tuple(phases))
    if key not in _CACHE:
        _CACHE[key] = _build_program(phases)
    return _CACHE[key]


def kernel(
    x, start_pos, freqs_cos, freqs_sin, mask, wq, wk, wv, wo, **_ignored
):
    nc = get_program()
    in_maps = _host_prep(x, freqs_cos, freqs_sin, mask, wq, wk, wv, wo)
    res = run_bass_kernel_spmd(nc, in_maps, core_ids=list(range(8)))
    partials = [res.results[c]["outp"] for c in range(8)]
    out = np.stack(
        [
            partials[b * G]
            + partials[b * G + 1]
            + partials[b * G + 2]
            + partials[b * G + 3]
            for b in range(B)
        ]
    ).astype(np.float32)
    return out
